# revision 5
# baseline (speedup 1.0000x reference)
"""GAT message-passing kernel for Trainium2, 8 NeuronCores.  (v3)

Problem (hardcoded): B=4, N=1024, H=F=O=G=128, E=16.
  features = concat([n_features, hidden], -1)            [B,N,256]
  values   = features @ W_m + b_m                        [B,N,128]
  logits   = att1 + att2^T + (e_features@w_ae) + att_g   [B,N,N]
  coefs    = softmax(leaky_relu(logits) + (adj-1)*1e9)
  out      = coefs @ values + features @ W_skip + b_skip

Sharding: 8 cores = (batch b = core//2) x (row half = core%2).

v3 strategy (v2 was dependency-stalled; DVE tree + ScalarE queue pollution):
  - host-side bf16 cast of ef/adj/features (halves HBM traffic vs fp32).
  - adj shipped as additive mask (adj-1)*1e9 bf16.
  - ef streamed as 2 MiB half-row-tile DMAs on the sync HWDGE queue.
  - DVE does ONLY mul-by-w + one pairwise add (e 16->8); the remaining
    e-reduction PLUS att2 broadcast PLUS mask add all happen on the PE as
    identity-matmul accumulation into PSUM (fp32 logit assembly).
  - ScalarE: leaky-relu (bias port carries att1+att_g+biases) from PSUM,
    exp with fused row-sum accumulator. ScalarE queue carries only the
    small DMAs; features load via dma_start_transpose (no PE transposes).
  - phase2 (A@V + skip + normalize + store) staggered one rt behind the
    streaming loop so no engine queue blocks on downstream results.
"""

import os
import numpy as np

B, N, H, F, E, G, O = 4, 1024, 128, 128, 16, 128, 128
DIN = F + H
NCORES = 8
ROWS = N // 2          # query rows per core
RT = ROWS // 128       # row tiles per core
KC = N // 128          # key chunks
KH = 2                 # k halves per row tile
KHW = N // KH

_cache = {}


def _build(stage=4):
    from contextlib import ExitStack
    import concourse.bacc as bacc
    import concourse.tile as tile
    import concourse.mybir as mybir
    import concourse.bass as bass

    fp32 = mybir.dt.float32
    bf16 = mybir.dt.bfloat16
    ALU = mybir.AluOpType
    AF = mybir.ActivationFunctionType

    nc = bacc.Bacc("TRN2", target_bir_lowering=False, debug=False,
                   num_devices=NCORES)

    ef_in = nc.dram_tensor("ef", [ROWS, N, E], bf16, kind="ExternalInput")
    adjm_in = nc.dram_tensor("adjm", [ROWS, N], bf16, kind="ExternalInput")
    nfk_in = nc.dram_tensor("nfk", [N, F], bf16, kind="ExternalInput")
    hidk_in = nc.dram_tensor("hidk", [N, H], bf16, kind="ExternalInput")
    nfr_in = nc.dram_tensor("nfr", [ROWS, F], bf16, kind="ExternalInput")
    hidr_in = nc.dram_tensor("hidr", [ROWS, H], bf16, kind="ExternalInput")
    g_in = nc.dram_tensor("g", [G, 1], bf16, kind="ExternalInput")
    Wm_in = nc.dram_tensor("Wm", [DIN, O], bf16, kind="ExternalInput")
    bm_in = nc.dram_tensor("bm", [1, O], bf16, kind="ExternalInput")
    Wsk_in = nc.dram_tensor("Wsk", [DIN, O], bf16, kind="ExternalInput")
    bsk_in = nc.dram_tensor("bsk", [1, O], bf16, kind="ExternalInput")
    wa12_in = nc.dram_tensor("wa12", [128, 4], bf16, kind="ExternalInput")
    wbc_in = nc.dram_tensor("wbc", [128, E], bf16, kind="ExternalInput")
    wag_in = nc.dram_tensor("wag", [G, 1], bf16, kind="ExternalInput")
    bs_in = nc.dram_tensor("bs", [1, 1], fp32, kind="ExternalInput")
    ident_in = nc.dram_tensor("ident", [128, 128], bf16, kind="ExternalInput")
    out_t = nc.dram_tensor("out", [ROWS, O], fp32, kind="ExternalOutput")

    with tile.TileContext(nc) as tc:
        with ExitStack() as ctx:
            singles = ctx.enter_context(tc.tile_pool(name="singles", bufs=1))
            efp = ctx.enter_context(tc.tile_pool(name="efp", bufs=4))
            work = ctx.enter_context(tc.tile_pool(name="work", bufs=2))
            small = ctx.enter_context(tc.tile_pool(name="small", bufs=2))
            psL = ctx.enter_context(tc.tile_pool(name="psL", bufs=1, space="PSUM"))
            psT = ctx.enter_context(tc.tile_pool(name="psT", bufs=2, space="PSUM"))
            psR = ctx.enter_context(tc.tile_pool(name="psR", bufs=2, space="PSUM"))

            # ---- queue priming: first ef half-DMAs go out before anything
            ef_tiles = {}
            for h in range(KH):
                t = efp.tile([128, KHW, E], bf16, tag="ef")
                nc.sync.dma_start(out=t, in_=ef_in[0:128, h * KHW:(h + 1) * KHW, :])
                ef_tiles[(0, h)] = t

            # ---- constants (scalar HWDGE queue) ------------------------
            w_tile = singles.tile([128, E], bf16)
            nc.scalar.dma_start(out=w_tile, in_=wbc_in.ap())
            ident_sb = singles.tile([128, 128], bf16)
            nc.scalar.dma_start(out=ident_sb, in_=ident_in.ap())
            ones_bf = singles.tile([1, 128], bf16)
            nc.vector.memset(ones_bf, 1.0)

            Wm_sb = singles.tile([128, 2, O], bf16)
            Wsk_sb = singles.tile([128, 2, O], bf16)
            for c in range(2):
                nc.scalar.dma_start(out=Wm_sb[:, c, :],
                                    in_=Wm_in[c * 128:(c + 1) * 128, :])
                nc.scalar.dma_start(out=Wsk_sb[:, c, :],
                                    in_=Wsk_in[c * 128:(c + 1) * 128, :])
            wa12_sb = singles.tile([128, 4], bf16)
            nc.scalar.dma_start(out=wa12_sb, in_=wa12_in.ap())
            bm_sb = singles.tile([1, O], bf16)
            nc.scalar.dma_start(out=bm_sb, in_=bm_in.ap())
            bsk_sb = singles.tile([1, O], bf16)
            nc.scalar.dma_start(out=bsk_sb, in_=bsk_in.ap())
            bs_sb = singles.tile([1, 1], fp32)
            nc.scalar.dma_start(out=bs_sb, in_=bs_in.ap())
            g_sb = singles.tile([128, 1], bf16)
            nc.scalar.dma_start(out=g_sb, in_=g_in.ap())
            wag_sb = singles.tile([128, 1], bf16)
            nc.scalar.dma_start(out=wag_sb, in_=wag_in.ap())

            # features^T via xbar DMA-transpose (HWDGE)
            fTk0 = singles.tile([128, N], bf16)
            fTk1 = singles.tile([128, N], bf16)
            fTr0 = singles.tile([128, ROWS], bf16)
            fTr1 = singles.tile([128, ROWS], bf16)
            nc.scalar.dma_start_transpose(out=fTk0, in_=nfk_in.ap())
            nc.scalar.dma_start_transpose(out=fTk1, in_=hidk_in.ap())
            nc.scalar.dma_start_transpose(out=fTr0, in_=nfr_in.ap())
            nc.scalar.dma_start_transpose(out=fTr1, in_=hidr_in.ap())

            # ---------------- phase 0: per-batch matmuls ----------------
            V = singles.tile([128, KC, O], bf16)
            for kc in range(KC):
                vps = psR.tile([128, O], fp32, tag="ret")
                ksl = slice(kc * 128, (kc + 1) * 128)
                nc.tensor.matmul(vps, fTk0[:, ksl], Wm_sb[:, 0, :],
                                 start=True, stop=False)
                nc.tensor.matmul(vps, fTk1[:, ksl], Wm_sb[:, 1, :],
                                 start=False, stop=False)
                nc.tensor.matmul(vps, ones_bf, bm_sb, start=False, stop=True)
                nc.vector.tensor_copy(V[:, kc, :], vps)

            att1_sb = singles.tile([128, RT], fp32)
            for rc in range(RT):
                aps = psR.tile([128, 1], fp32, tag="ret")
                rsl = slice(rc * 128, (rc + 1) * 128)
                nc.tensor.matmul(aps, fTr0[:, rsl], wa12_sb[:, 0:1],
                                 start=True, stop=False)
                nc.tensor.matmul(aps, fTr1[:, rsl], wa12_sb[:, 1:2],
                                 start=False, stop=True)
                nc.vector.tensor_copy(att1_sb[:, rc:rc + 1], aps)

            # att_g + sum(b_a*): sc = g@wag + bs -> bcast [128,1]
            gps = psR.tile([1, 1], fp32, tag="ret")
            nc.tensor.matmul(gps, g_sb, wag_sb, start=True, stop=True)
            sc1 = singles.tile([1, 1], fp32)
            nc.vector.tensor_copy(sc1, gps)
            nc.vector.tensor_scalar_add(sc1, sc1, bs_sb)
            sc1b = singles.tile([1, 1], bf16)
            nc.vector.tensor_copy(sc1b, sc1)
            scps = psR.tile([128, 1], fp32, tag="ret")
            nc.tensor.matmul(scps, ones_bf, sc1b, start=True, stop=True)
            sc128 = singles.tile([128, 1], fp32)
            nc.vector.tensor_copy(sc128, scps)
            att1g = singles.tile([128, RT], fp32)
            nc.vector.tensor_scalar_add(att1g, att1_sb, sc128)

            # att2 row over all keys: [1, N] bf16
            att2row = singles.tile([1, N], bf16)
            for khf in range(2):
                a2ps = psR.tile([1, 512], fp32, tag="ret")
                ksl = slice(khf * 512, (khf + 1) * 512)
                nc.tensor.matmul(a2ps, wa12_sb[:, 2:3], fTk0[:, ksl],
                                 start=True, stop=False)
                nc.tensor.matmul(a2ps, wa12_sb[:, 3:4], fTk1[:, ksl],
                                 start=False, stop=True)
                nc.vector.tensor_copy(att2row[:, ksl], a2ps)

            # ---------------- phase 1+2: staggered pipeline -------------
            sim_leaky = bool(os.environ.get("GAT_SIM_LEAKY"))
            ph2_state = {}

            def phase1(rt):
                rsl = slice(rt * 128, (rt + 1) * 128)
                for h in range(KH):
                    if (rt, h) not in ef_tiles:
                        t = efp.tile([128, KHW, E], bf16, tag="ef")
                        nc.sync.dma_start(
                            out=t, in_=ef_in[rsl, h * KHW:(h + 1) * KHW, :])
                        ef_tiles[(rt, h)] = t
                adjm_t = work.tile([128, N], bf16, tag="adj")
                nc.scalar.dma_start(out=adjm_t, in_=adjm_in[rsl, :])

                lk = work.tile([128, N], bf16, tag="lk")
                wta = w_tile[:]
                wpat = bass.AP(tensor=wta.tensor, offset=wta.offset,
                               ap=[list(wta.ap[0]), [0, KHW], [1, E]])
                for h in range(KH):
                    ef_t = ef_tiles.pop((rt, h))
                    hsl = slice(h * KHW, (h + 1) * KHW)
                    # DVE: in-place mul by w, then e 16->8 pairwise add
                    nc.vector.tensor_mul(ef_t, ef_t, wpat)
                    nc.vector.tensor_add(ef_t[:, :, 0:8], ef_t[:, :, 0:8],
                                         ef_t[:, :, 8:16])
                    # PE: logit assembly in PSUM fp32:
                    #   sum_j wef[:,:,j] + bcast(att2row) + adjm
                    lps = psL.tile([128, KHW], fp32, tag=f"l{h}")
                    for j in range(8):
                        nc.tensor.matmul(lps, ident_sb, ef_t[:, :, j],
                                         start=(j == 0), stop=False)
                    nc.tensor.matmul(lps, ident_sb, adjm_t[:, hsl],
                                     start=False, stop=False)
                    nc.tensor.matmul(lps, ones_bf, att2row[:, hsl],
                                     start=False, stop=True)
                    # ScalarE: leaky_relu(x + att1g) from PSUM -> SBUF bf16
                    if sim_leaky:
                        l2 = work.tile([128, KHW], fp32, tag=f"l2{h}")
                        nc.vector.tensor_scalar_add(l2, lps,
                                                    att1g[:, rt:rt + 1])
                        nc.vector.scalar_tensor_tensor(
                            out=lk[:, hsl], in0=l2, scalar=0.01, in1=l2,
                            op0=ALU.mult, op1=ALU.max)
                    else:
                        nc.scalar.activation(lk[:, hsl], lps, AF.Lrelu,
                                             bias=att1g[:, rt:rt + 1],
                                             alpha=0.01)
                ex = work.tile([128, N], bf16, tag="ex")
                s = small.tile([128, 1], fp32, tag="s")
                nc.scalar.activation(ex, lk, AF.Exp, accum_out=s)
                ph2_state[rt] = (ex, s)

            def phase2(rt):
                rsl = slice(rt * 128, (rt + 1) * 128)
                ex, s = ph2_state.pop(rt)
                if stage == 1:
                    o1 = work.tile([128, O], fp32, tag="outsb")
                    nc.vector.tensor_copy(o1, ex[:, :O])
                    nc.sync.dma_start(out=out_t[rsl, :], in_=o1)
                    return
                ret_ps = psR.tile([128, O], fp32, tag="ret")
                for kc in range(KC):
                    tp = psT.tile([128, 128], bf16, tag="tp1")
                    nc.tensor.transpose(tp, ex[:, kc * 128:(kc + 1) * 128],
                                        ident_sb)
                    ctT = small.tile([128, 128], bf16, tag="ctT")
                    nc.scalar.copy(out=ctT, in_=tp)
                    nc.tensor.matmul(ret_ps, ctT, V[:, kc, :],
                                     start=(kc == 0), stop=(kc == KC - 1))
                sk_ps = psT.tile([128, O], fp32, tag="skp")
                nc.tensor.matmul(sk_ps, fTr0[:, rsl], Wsk_sb[:, 0, :],
                                 start=True, stop=False)
                nc.tensor.matmul(sk_ps, fTr1[:, rsl], Wsk_sb[:, 1, :],
                                 start=False, stop=False)
                nc.tensor.matmul(sk_ps, ones_bf, bsk_sb,
                                 start=False, stop=True)
                r = small.tile([128, 1], fp32, tag="r")
                nc.vector.reciprocal(r, s)
                rets = small.tile([128, O], fp32, tag="rets")
                nc.scalar.mul(rets, ret_ps, r)
                sks = small.tile([128, O], fp32, tag="sks")
                nc.scalar.copy(out=sks, in_=sk_ps)
                out_sb = work.tile([128, O], fp32, tag="outsb")
                nc.vector.tensor_add(out_sb, rets, sks)
                nc.sync.dma_start(out=out_t[rsl, :], in_=out_sb)

            if stage >= 1:
                for rt in range(RT):
                    phase1(rt)
                    if rt >= 1:
                        phase2(rt - 1)
                phase2(RT - 1)
            else:
                for rt in range(RT):
                    o0 = work.tile([128, O], fp32, tag="outsb")
                    nc.vector.tensor_copy(o0, V[:, rt, :])
                    nc.sync.dma_start(out=out_t[rt * 128:(rt + 1) * 128, :],
                                      in_=o0)

    nc.compile()
    return nc


def _get_nc():
    if "nc" not in _cache:
        _cache["nc"] = _build(stage=int(os.environ.get("GAT_STAGE", "4")))
    return _cache["nc"]


def _in_maps(hidden, n_features, e_features, g_features, adj,
             W_m, b_m, W_skip, b_skip, w_a1, b_a1, w_a2, b_a2,
             w_ae, b_ae, w_ag, b_ag):
    import ml_dtypes
    bf16 = ml_dtypes.bfloat16
    f32 = np.float32
    asb = lambda x: np.ascontiguousarray(np.asarray(x).astype(bf16))
    bsum = (np.float32(np.asarray(b_a1).reshape(())) +
            np.float32(np.asarray(b_a2).reshape(())) +
            np.float32(np.asarray(b_ae).reshape(())) +
            np.float32(np.asarray(b_ag).reshape(())))
    wbc = np.broadcast_to(np.asarray(w_ae, f32).reshape(1, E), (128, E))
    # wa12 columns: [wa1_chunk0, wa1_chunk1, wa2_chunk0, wa2_chunk1]
    wa12 = np.stack([np.asarray(w_a1, f32).reshape(2, 128)[0],
                     np.asarray(w_a1, f32).reshape(2, 128)[1],
                     np.asarray(w_a2, f32).reshape(2, 128)[0],
                     np.asarray(w_a2, f32).reshape(2, 128)[1]], axis=1)
    shared = {
        "Wm": asb(W_m), "bm": asb(b_m).reshape(1, O),
        "Wsk": asb(W_skip), "bsk": asb(b_skip).reshape(1, O),
        "wa12": asb(wa12), "wbc": asb(wbc), "wag": asb(w_ag),
        "bs": np.array([[bsum]], dtype=f32),
        "ident": np.eye(128, dtype=f32).astype(bf16),
    }
    maps = []
    adjm_all = {}
    for c in range(NCORES):
        b, h = c // 2, c % 2
        rows = slice(h * ROWS, (h + 1) * ROWS)
        if b not in adjm_all:
            adjm_all[b] = ((np.asarray(adj[b], f32) - 1.0) *
                           np.float32(1e9)).astype(bf16)
        m = dict(shared)
        m["ef"] = asb(e_features[b, rows])
        m["adjm"] = np.ascontiguousarray(adjm_all[b][rows])
        m["nfk"] = asb(n_features[b])
        m["hidk"] = asb(hidden[b])
        m["nfr"] = asb(n_features[b][rows])
        m["hidr"] = asb(hidden[b][rows])
        m["g"] = asb(g_features[b]).reshape(G, 1)
        maps.append(m)
    return maps


def kernel(hidden, n_features, e_features, g_features, adj,
           W_m, b_m, W_skip, b_skip, w_a1, b_a1, w_a2, b_a2,
           w_ae, b_ae, w_ag, b_ag):
    from concourse import bass_utils
    nc = _get_nc()
    maps = _in_maps(hidden, n_features, e_features, g_features, adj,
                    W_m, b_m, W_skip, b_skip, w_a1, b_a1, w_a2, b_a2,
                    w_ae, b_ae, w_ag, b_ag)
    res = bass_utils.run_bass_kernel_spmd(nc, maps, core_ids=list(range(NCORES)))
    out = np.empty((B, N, O), np.float32)
    for c in range(NCORES):
        b, h = c // 2, c % 2
        out[b, h * ROWS:(h + 1) * ROWS] = res.results[c]["out"]
    return out


# revision 7
# speedup vs baseline: 1.1775x; 1.1775x over previous
"""GAT message-passing kernel for Trainium2, 8 NeuronCores.  (v3)

Problem (hardcoded): B=4, N=1024, H=F=O=G=128, E=16.
  features = concat([n_features, hidden], -1)            [B,N,256]
  values   = features @ W_m + b_m                        [B,N,128]
  logits   = att1 + att2^T + (e_features@w_ae) + att_g   [B,N,N]
  coefs    = softmax(leaky_relu(logits) + (adj-1)*1e9)
  out      = coefs @ values + features @ W_skip + b_skip

Sharding: 8 cores = (batch b = core//2) x (row half = core%2).

v3 strategy (v2 was dependency-stalled; DVE tree + ScalarE queue pollution):
  - host-side bf16 cast of ef/adj/features (halves HBM traffic vs fp32).
  - adj shipped as additive mask (adj-1)*1e9 bf16.
  - ef streamed as 2 MiB half-row-tile DMAs on the sync HWDGE queue.
  - DVE does ONLY mul-by-w + one pairwise add (e 16->8); the remaining
    e-reduction PLUS att2 broadcast PLUS mask add all happen on the PE as
    identity-matmul accumulation into PSUM (fp32 logit assembly).
  - ScalarE: leaky-relu (bias port carries att1+att_g+biases) from PSUM,
    exp with fused row-sum accumulator. ScalarE queue carries only the
    small DMAs; features load via dma_start_transpose (no PE transposes).
  - phase2 (A@V + skip + normalize + store) staggered one rt behind the
    streaming loop so no engine queue blocks on downstream results.
"""

import os
import numpy as np

B, N, H, F, E, G, O = 4, 1024, 128, 128, 16, 128, 128
DIN = F + H
NCORES = 8
ROWS = N // 2          # query rows per core
RT = ROWS // 128       # row tiles per core
KC = N // 128          # key chunks
KH = 2                 # k halves per row tile
KHW = N // KH

_cache = {}


def _build(stage=4):
    from contextlib import ExitStack
    import concourse.bacc as bacc
    import concourse.tile as tile
    import concourse.mybir as mybir
    import concourse.bass as bass

    fp32 = mybir.dt.float32
    bf16 = mybir.dt.bfloat16
    ALU = mybir.AluOpType
    AF = mybir.ActivationFunctionType

    nc = bacc.Bacc("TRN2", target_bir_lowering=False, debug=False,
                   num_devices=NCORES)

    ef_in = nc.dram_tensor("ef", [ROWS, N, E], bf16, kind="ExternalInput")
    adjm_in = nc.dram_tensor("adjm", [ROWS, N], bf16, kind="ExternalInput")
    nfk_in = nc.dram_tensor("nfk", [N, F], bf16, kind="ExternalInput")
    hidk_in = nc.dram_tensor("hidk", [N, H], bf16, kind="ExternalInput")
    nfr_in = nc.dram_tensor("nfr", [ROWS, F], bf16, kind="ExternalInput")
    hidr_in = nc.dram_tensor("hidr", [ROWS, H], bf16, kind="ExternalInput")
    g_in = nc.dram_tensor("g", [G, 1], bf16, kind="ExternalInput")
    Wm_in = nc.dram_tensor("Wm", [DIN, O], bf16, kind="ExternalInput")
    bm_in = nc.dram_tensor("bm", [1, O], bf16, kind="ExternalInput")
    Wsk_in = nc.dram_tensor("Wsk", [DIN, O], bf16, kind="ExternalInput")
    bsk_in = nc.dram_tensor("bsk", [1, O], bf16, kind="ExternalInput")
    wa12_in = nc.dram_tensor("wa12", [128, 4], bf16, kind="ExternalInput")
    wbc_in = nc.dram_tensor("wbc", [128, E], bf16, kind="ExternalInput")
    wag_in = nc.dram_tensor("wag", [G, 1], bf16, kind="ExternalInput")
    bs_in = nc.dram_tensor("bs", [1, 1], fp32, kind="ExternalInput")
    ident_in = nc.dram_tensor("ident", [128, 128], bf16, kind="ExternalInput")
    out_t = nc.dram_tensor("out", [ROWS, O], fp32, kind="ExternalOutput")

    with tile.TileContext(nc) as tc:
        with ExitStack() as ctx:
            singles = ctx.enter_context(tc.tile_pool(name="singles", bufs=1))
            efp = ctx.enter_context(tc.tile_pool(name="efp", bufs=4))
            work = ctx.enter_context(tc.tile_pool(name="work", bufs=2))
            small = ctx.enter_context(tc.tile_pool(name="small", bufs=2))
            psL = ctx.enter_context(tc.tile_pool(name="psL", bufs=1, space="PSUM"))
            psT = ctx.enter_context(tc.tile_pool(name="psT", bufs=2, space="PSUM"))
            psR = ctx.enter_context(tc.tile_pool(name="psR", bufs=2, space="PSUM"))

            # ---- queue priming: first ef half-DMAs go out before anything
            ef_tiles = {}
            for h in range(KH):
                t = efp.tile([128, KHW, E], bf16, tag="ef")
                nc.sync.dma_start(out=t, in_=ef_in[0:128, h * KHW:(h + 1) * KHW, :])
                ef_tiles[(0, h)] = t

            # ---- constants (scalar HWDGE queue) ------------------------
            w_tile = singles.tile([128, E], bf16)
            nc.scalar.dma_start(out=w_tile, in_=wbc_in.ap())
            ident_sb = singles.tile([128, 128], bf16)
            nc.scalar.dma_start(out=ident_sb, in_=ident_in.ap())
            ones_bf = singles.tile([1, 128], bf16)
            nc.vector.memset(ones_bf, 1.0)

            Wm_sb = singles.tile([128, 2, O], bf16)
            Wsk_sb = singles.tile([128, 2, O], bf16)
            for c in range(2):
                nc.scalar.dma_start(out=Wm_sb[:, c, :],
                                    in_=Wm_in[c * 128:(c + 1) * 128, :])
                nc.scalar.dma_start(out=Wsk_sb[:, c, :],
                                    in_=Wsk_in[c * 128:(c + 1) * 128, :])
            wa12_sb = singles.tile([128, 4], bf16)
            nc.scalar.dma_start(out=wa12_sb, in_=wa12_in.ap())
            bm_sb = singles.tile([1, O], bf16)
            nc.scalar.dma_start(out=bm_sb, in_=bm_in.ap())
            bsk_sb = singles.tile([1, O], bf16)
            nc.scalar.dma_start(out=bsk_sb, in_=bsk_in.ap())
            bs_sb = singles.tile([1, 1], fp32)
            nc.scalar.dma_start(out=bs_sb, in_=bs_in.ap())
            g_sb = singles.tile([128, 1], bf16)
            nc.scalar.dma_start(out=g_sb, in_=g_in.ap())
            wag_sb = singles.tile([128, 1], bf16)
            nc.scalar.dma_start(out=wag_sb, in_=wag_in.ap())

            # features^T via xbar DMA-transpose (HWDGE)
            fTk0 = singles.tile([128, N], bf16)
            fTk1 = singles.tile([128, N], bf16)
            fTr0 = singles.tile([128, ROWS], bf16)
            fTr1 = singles.tile([128, ROWS], bf16)
            nc.sync.dma_start_transpose(out=fTk0, in_=nfk_in.ap())
            nc.sync.dma_start_transpose(out=fTk1, in_=hidk_in.ap())
            nc.sync.dma_start_transpose(out=fTr0, in_=nfr_in.ap())
            nc.sync.dma_start_transpose(out=fTr1, in_=hidr_in.ap())

            # ---------------- phase 0: per-batch matmuls ----------------
            V = singles.tile([128, KC, O], bf16)
            for kc in range(KC):
                vps = psR.tile([128, O], fp32, tag="ret")
                ksl = slice(kc * 128, (kc + 1) * 128)
                nc.tensor.matmul(vps, fTk0[:, ksl], Wm_sb[:, 0, :],
                                 start=True, stop=False)
                nc.tensor.matmul(vps, fTk1[:, ksl], Wm_sb[:, 1, :],
                                 start=False, stop=False)
                nc.tensor.matmul(vps, ones_bf, bm_sb, start=False, stop=True)
                nc.vector.tensor_copy(V[:, kc, :], vps)

            att1_sb = singles.tile([128, RT], fp32)
            for rc in range(RT):
                aps = psR.tile([128, 1], fp32, tag="ret")
                rsl = slice(rc * 128, (rc + 1) * 128)
                nc.tensor.matmul(aps, fTr0[:, rsl], wa12_sb[:, 0:1],
                                 start=True, stop=False)
                nc.tensor.matmul(aps, fTr1[:, rsl], wa12_sb[:, 1:2],
                                 start=False, stop=True)
                nc.vector.tensor_copy(att1_sb[:, rc:rc + 1], aps)

            # att_g + sum(b_a*): sc = g@wag + bs -> bcast [128,1]
            gps = psR.tile([1, 1], fp32, tag="ret")
            nc.tensor.matmul(gps, g_sb, wag_sb, start=True, stop=True)
            sc1 = singles.tile([1, 1], fp32)
            nc.vector.tensor_copy(sc1, gps)
            nc.vector.tensor_scalar_add(sc1, sc1, bs_sb)
            sc1b = singles.tile([1, 1], bf16)
            nc.vector.tensor_copy(sc1b, sc1)
            scps = psR.tile([128, 1], fp32, tag="ret")
            nc.tensor.matmul(scps, ones_bf, sc1b, start=True, stop=True)
            sc128 = singles.tile([128, 1], fp32)
            nc.vector.tensor_copy(sc128, scps)
            att1g = singles.tile([128, RT], fp32)
            nc.vector.tensor_scalar_add(att1g, att1_sb, sc128)

            # att2 row over all keys: [1, N] bf16
            att2row = singles.tile([1, N], bf16)
            for khf in range(2):
                a2ps = psR.tile([1, 512], fp32, tag="ret")
                ksl = slice(khf * 512, (khf + 1) * 512)
                nc.tensor.matmul(a2ps, wa12_sb[:, 2:3], fTk0[:, ksl],
                                 start=True, stop=False)
                nc.tensor.matmul(a2ps, wa12_sb[:, 3:4], fTk1[:, ksl],
                                 start=False, stop=True)
                nc.vector.tensor_copy(att2row[:, ksl], a2ps)

            # ---------------- phase 1+2: staggered pipeline -------------
            sim_leaky = bool(os.environ.get("GAT_SIM_LEAKY"))
            ph2_state = {}

            def phase1(rt):
                rsl = slice(rt * 128, (rt + 1) * 128)
                for h in range(KH):
                    if (rt, h) not in ef_tiles:
                        t = efp.tile([128, KHW, E], bf16, tag="ef")
                        nc.sync.dma_start(
                            out=t, in_=ef_in[rsl, h * KHW:(h + 1) * KHW, :])
                        ef_tiles[(rt, h)] = t
                adjm_t = work.tile([128, N], bf16, tag="adj")
                nc.scalar.dma_start(out=adjm_t, in_=adjm_in[rsl, :])

                lk = work.tile([128, N], bf16, tag="lk")
                wta = w_tile[:]
                wpat = bass.AP(tensor=wta.tensor, offset=wta.offset,
                               ap=[list(wta.ap[0]), [0, KHW], [1, E]])
                acc_b = work.tile([128, N], bf16, tag="acc_b")
                for h in range(KH):
                    ef_t = ef_tiles.pop((rt, h))
                    hsl = slice(h * KHW, (h + 1) * KHW)
                    # DVE: in-place mul by w, then e 16->8 pairwise add
                    nc.vector.tensor_mul(ef_t, ef_t, wpat)
                    nc.vector.tensor_add(ef_t[:, :, 0:8], ef_t[:, :, 0:8],
                                         ef_t[:, :, 8:16])
                    # GpSimd: finish the e-tree and gather contiguously
                    nc.gpsimd.tensor_add(ef_t[:, :, 0:4], ef_t[:, :, 0:4],
                                         ef_t[:, :, 4:8])
                    nc.gpsimd.tensor_add(ef_t[:, :, 0:2], ef_t[:, :, 0:2],
                                         ef_t[:, :, 2:4])
                    nc.gpsimd.tensor_add(acc_b[:, hsl], ef_t[:, :, 0],
                                         ef_t[:, :, 1])
                    # PE: logit assembly in PSUM fp32 (all-contiguous rhs)
                    lps = psL.tile([128, KHW], fp32, tag=f"l{h}")
                    nc.tensor.matmul(lps, ident_sb, acc_b[:, hsl],
                                     start=True, stop=False)
                    nc.tensor.matmul(lps, ident_sb, adjm_t[:, hsl],
                                     start=False, stop=False)
                    nc.tensor.matmul(lps, ones_bf, att2row[:, hsl],
                                     start=False, stop=True)
                    # ScalarE: leaky_relu(x + att1g) from PSUM -> SBUF bf16
                    if sim_leaky:
                        l2 = work.tile([128, KHW], fp32, tag=f"l2{h}")
                        nc.vector.tensor_scalar_add(l2, lps,
                                                    att1g[:, rt:rt + 1])
                        nc.vector.scalar_tensor_tensor(
                            out=lk[:, hsl], in0=l2, scalar=0.01, in1=l2,
                            op0=ALU.mult, op1=ALU.max)
                    else:
                        nc.scalar.activation(lk[:, hsl], lps, AF.Lrelu,
                                             bias=att1g[:, rt:rt + 1],
                                             alpha=0.01)
                ex = work.tile([128, N], bf16, tag="ex")
                s = small.tile([128, 1], fp32, tag="s")
                nc.scalar.activation(ex, lk, AF.Exp, accum_out=s)
                ph2_state[rt] = (ex, s)

            def phase2(rt):
                rsl = slice(rt * 128, (rt + 1) * 128)
                ex, s = ph2_state.pop(rt)
                if stage == 1:
                    o1 = work.tile([128, O], fp32, tag="outsb")
                    nc.vector.tensor_copy(o1, ex[:, :O])
                    nc.sync.dma_start(out=out_t[rsl, :], in_=o1)
                    return
                ret_ps = psR.tile([128, O], fp32, tag="ret")
                for kc in range(KC):
                    tp = psT.tile([128, 128], bf16, tag="tp1")
                    nc.tensor.transpose(tp, ex[:, kc * 128:(kc + 1) * 128],
                                        ident_sb)
                    ctT = small.tile([128, 128], bf16, tag="ctT")
                    nc.scalar.copy(out=ctT, in_=tp)
                    nc.tensor.matmul(ret_ps, ctT, V[:, kc, :],
                                     start=(kc == 0), stop=(kc == KC - 1))
                sk_ps = psT.tile([128, O], fp32, tag="skp")
                nc.tensor.matmul(sk_ps, fTr0[:, rsl], Wsk_sb[:, 0, :],
                                 start=True, stop=False)
                nc.tensor.matmul(sk_ps, fTr1[:, rsl], Wsk_sb[:, 1, :],
                                 start=False, stop=False)
                nc.tensor.matmul(sk_ps, ones_bf, bsk_sb,
                                 start=False, stop=True)
                r = small.tile([128, 1], fp32, tag="r")
                nc.vector.reciprocal(r, s)
                rets = small.tile([128, O], fp32, tag="rets")
                nc.scalar.mul(rets, ret_ps, r)
                sks = small.tile([128, O], fp32, tag="sks")
                nc.scalar.copy(out=sks, in_=sk_ps)
                out_sb = work.tile([128, O], fp32, tag="outsb")
                nc.vector.tensor_add(out_sb, rets, sks)
                nc.sync.dma_start(out=out_t[rsl, :], in_=out_sb)

            if stage >= 1:
                for rt in range(RT):
                    phase1(rt)
                    if rt >= 1:
                        phase2(rt - 1)
                phase2(RT - 1)
            else:
                for rt in range(RT):
                    o0 = work.tile([128, O], fp32, tag="outsb")
                    nc.vector.tensor_copy(o0, V[:, rt, :])
                    nc.sync.dma_start(out=out_t[rt * 128:(rt + 1) * 128, :],
                                      in_=o0)

    nc.compile()
    return nc


def _get_nc():
    if "nc" not in _cache:
        _cache["nc"] = _build(stage=int(os.environ.get("GAT_STAGE", "4")))
    return _cache["nc"]


def _in_maps(hidden, n_features, e_features, g_features, adj,
             W_m, b_m, W_skip, b_skip, w_a1, b_a1, w_a2, b_a2,
             w_ae, b_ae, w_ag, b_ag):
    import ml_dtypes
    bf16 = ml_dtypes.bfloat16
    f32 = np.float32
    asb = lambda x: np.ascontiguousarray(np.asarray(x).astype(bf16))
    bsum = (np.float32(np.asarray(b_a1).reshape(())) +
            np.float32(np.asarray(b_a2).reshape(())) +
            np.float32(np.asarray(b_ae).reshape(())) +
            np.float32(np.asarray(b_ag).reshape(())))
    wbc = np.broadcast_to(np.asarray(w_ae, f32).reshape(1, E), (128, E))
    # wa12 columns: [wa1_chunk0, wa1_chunk1, wa2_chunk0, wa2_chunk1]
    wa12 = np.stack([np.asarray(w_a1, f32).reshape(2, 128)[0],
                     np.asarray(w_a1, f32).reshape(2, 128)[1],
                     np.asarray(w_a2, f32).reshape(2, 128)[0],
                     np.asarray(w_a2, f32).reshape(2, 128)[1]], axis=1)
    shared = {
        "Wm": asb(W_m), "bm": asb(b_m).reshape(1, O),
        "Wsk": asb(W_skip), "bsk": asb(b_skip).reshape(1, O),
        "wa12": asb(wa12), "wbc": asb(wbc), "wag": asb(w_ag),
        "bs": np.array([[bsum]], dtype=f32),
        "ident": np.eye(128, dtype=f32).astype(bf16),
    }
    maps = []
    adjm_all = {}
    for c in range(NCORES):
        b, h = c // 2, c % 2
        rows = slice(h * ROWS, (h + 1) * ROWS)
        if b not in adjm_all:
            adjm_all[b] = ((np.asarray(adj[b], f32) - 1.0) *
                           np.float32(1e9)).astype(bf16)
        m = dict(shared)
        m["ef"] = asb(e_features[b, rows])
        m["adjm"] = np.ascontiguousarray(adjm_all[b][rows])
        m["nfk"] = asb(n_features[b])
        m["hidk"] = asb(hidden[b])
        m["nfr"] = asb(n_features[b][rows])
        m["hidr"] = asb(hidden[b][rows])
        m["g"] = asb(g_features[b]).reshape(G, 1)
        maps.append(m)
    return maps


def kernel(hidden, n_features, e_features, g_features, adj,
           W_m, b_m, W_skip, b_skip, w_a1, b_a1, w_a2, b_a2,
           w_ae, b_ae, w_ag, b_ag):
    from concourse import bass_utils
    nc = _get_nc()
    maps = _in_maps(hidden, n_features, e_features, g_features, adj,
                    W_m, b_m, W_skip, b_skip, w_a1, b_a1, w_a2, b_a2,
                    w_ae, b_ae, w_ag, b_ag)
    res = bass_utils.run_bass_kernel_spmd(nc, maps, core_ids=list(range(NCORES)))
    out = np.empty((B, N, O), np.float32)
    for c in range(NCORES):
        b, h = c // 2, c % 2
        out[b, h * ROWS:(h + 1) * ROWS] = res.results[c]["out"]
    return out


# revision 9
# speedup vs baseline: 1.5821x; 1.3437x over previous
"""GAT message-passing kernel for Trainium2, 8 NeuronCores.  (v5)

Problem (hardcoded): B=4, N=1024, H=F=O=G=128, E=16.
  features = concat([n_features, hidden], -1)            [B,N,256]
  values   = features @ W_m + b_m                        [B,N,128]
  logits   = att1 + att2^T + (e_features@w_ae) + att_g   [B,N,N]
  coefs    = softmax(leaky_relu(logits) + (adj-1)*1e9)
  out      = coefs @ values + features @ W_skip + b_skip

Sharding: 8 cores = (batch b = core//2) x (row half = core%2).

v5 strategy: the e-feature contraction runs entirely on the TENSOR engine.
  Host permutes ef into T[p=(rm*4+es), kh, g, s, k'] (rm=row%32, g=row
  group of 32, e=4s+es) so that a col-tiled matmul with a constant
  block-diagonal stationary Wd_s[128,32] (w_ae folded in) computes
  att_e for rows 32g..32g+31 directly at their natural partitions:
    lps[32g+rm', k] += sum_{rm,es} Wd_s[(rm,es),rm'] * T[(rm,es),g,s,k]
  PSUM accumulates att_e + adj mask + att2^T in fp32; att1+att_g ride the
  ScalarE leaky-relu bias port.  DVE does (almost) nothing per tile;
  GpSimd unused (its SBUF port contends with DVE).  DMA: bf16 host-cast,
  2 MiB k-half transfers on the sync HWDGE queue; small loads on scalar;
  features^T via xbar dma transpose.
"""

import os
import numpy as np

B, N, H, F, E, G, O = 4, 1024, 128, 128, 16, 128, 128
DIN = F + H
NCORES = 8
ROWS = N // 2          # query rows per core
RT = ROWS // 128       # row tiles per core
KC = N // 128          # key chunks
KH = 2                 # k halves
KHW = N // KH          # 512
GPT = 4                # 32-row groups per row tile
NS = 4                 # e-channel subgroups (4 channels each)

_cache = {}


def _build(stage=4):
    from contextlib import ExitStack
    import concourse.bacc as bacc
    import concourse.tile as tile
    import concourse.mybir as mybir
    import concourse.bass as bass

    fp32 = mybir.dt.float32
    bf16 = mybir.dt.bfloat16
    ALU = mybir.AluOpType
    AF = mybir.ActivationFunctionType

    nc = bacc.Bacc("TRN2", target_bir_lowering=False, debug=False,
                   num_devices=NCORES)

    # T layout: [p=(rm*4+es), kh, G, s, k'] ; G = global 32-row group
    T_in = nc.dram_tensor("T", [128, KH, 16, NS, KHW], bf16,
                          kind="ExternalInput")
    Wd_in = nc.dram_tensor("Wd", [128, NS, 32], bf16, kind="ExternalInput")
    adj_in = nc.dram_tensor("adj", [ROWS, N], bf16, kind="ExternalInput")
    nfk_in = nc.dram_tensor("nfk", [N, F], bf16, kind="ExternalInput")
    hidk_in = nc.dram_tensor("hidk", [N, H], bf16, kind="ExternalInput")
    nfr_in = nc.dram_tensor("nfr", [ROWS, F], bf16, kind="ExternalInput")
    hidr_in = nc.dram_tensor("hidr", [ROWS, H], bf16, kind="ExternalInput")
    g_in = nc.dram_tensor("g", [G, 1], bf16, kind="ExternalInput")
    Wm_in = nc.dram_tensor("Wm", [DIN, O], bf16, kind="ExternalInput")
    bm_in = nc.dram_tensor("bm", [1, O], bf16, kind="ExternalInput")
    Wsk_in = nc.dram_tensor("Wsk", [DIN, O], bf16, kind="ExternalInput")
    bsk_in = nc.dram_tensor("bsk", [1, O], bf16, kind="ExternalInput")
    wa12_in = nc.dram_tensor("wa12", [128, 4], bf16, kind="ExternalInput")
    wag_in = nc.dram_tensor("wag", [G, 1], bf16, kind="ExternalInput")
    bs_in = nc.dram_tensor("bs", [1, 1], fp32, kind="ExternalInput")
    ident_in = nc.dram_tensor("ident", [128, 128], bf16, kind="ExternalInput")
    out_t = nc.dram_tensor("out", [ROWS, O], fp32, kind="ExternalOutput")

    with tile.TileContext(nc) as tc:
        with ExitStack() as ctx:
            singles = ctx.enter_context(tc.tile_pool(name="singles", bufs=1))
            efp = ctx.enter_context(tc.tile_pool(name="efp", bufs=6))
            work = ctx.enter_context(tc.tile_pool(name="work", bufs=2))
            small = ctx.enter_context(tc.tile_pool(name="small", bufs=2))
            psL = ctx.enter_context(tc.tile_pool(name="psL", bufs=1, space="PSUM"))
            psT = ctx.enter_context(tc.tile_pool(name="psT", bufs=2, space="PSUM"))
            psR = ctx.enter_context(tc.tile_pool(name="psR", bufs=2, space="PSUM"))

            # ---- prime the ef stream (split first transfer for fast start)
            ef_tiles = {}
            t00 = efp.tile([128, GPT, NS, KHW], bf16, tag="ef")
            nc.sync.dma_start(out=t00[:, 0:2, :, :], in_=T_in[:, 0, 0:2, :, :])
            nc.sync.dma_start(out=t00[:, 2:4, :, :], in_=T_in[:, 0, 2:4, :, :])
            ef_tiles[(0, 0)] = t00
            t01 = efp.tile([128, GPT, NS, KHW], bf16, tag="ef")
            nc.sync.dma_start(out=t01, in_=T_in[:, 1, 0:GPT, :, :])
            ef_tiles[(0, 1)] = t01

            # ---- constants (scalar HWDGE queue) ------------------------
            Wd_sb = singles.tile([128, NS, 32], bf16)
            nc.scalar.dma_start(out=Wd_sb, in_=Wd_in.ap())
            adj_tiles = {}
            adj0 = work.tile([128, N], bf16, tag="adj")
            nc.scalar.dma_start(out=adj0, in_=adj_in[0:128, :])
            adj_tiles[0] = adj0
            ident_sb = singles.tile([128, 128], bf16)
            nc.scalar.dma_start(out=ident_sb, in_=ident_in.ap())
            ones_bf = singles.tile([1, 128], bf16)
            nc.vector.memset(ones_bf, 1.0)

            Wm_sb = singles.tile([128, 2, O], bf16)
            Wsk_sb = singles.tile([128, 2, O], bf16)
            for c in range(2):
                nc.scalar.dma_start(out=Wm_sb[:, c, :],
                                    in_=Wm_in[c * 128:(c + 1) * 128, :])
                nc.scalar.dma_start(out=Wsk_sb[:, c, :],
                                    in_=Wsk_in[c * 128:(c + 1) * 128, :])
            wa12_sb = singles.tile([128, 4], bf16)
            nc.scalar.dma_start(out=wa12_sb, in_=wa12_in.ap())
            bm_sb = singles.tile([1, O], bf16)
            nc.scalar.dma_start(out=bm_sb, in_=bm_in.ap())
            bsk_sb = singles.tile([1, O], bf16)
            nc.scalar.dma_start(out=bsk_sb, in_=bsk_in.ap())
            bs_sb = singles.tile([1, 1], fp32)
            nc.scalar.dma_start(out=bs_sb, in_=bs_in.ap())
            g_sb = singles.tile([128, 1], bf16)
            nc.scalar.dma_start(out=g_sb, in_=g_in.ap())
            wag_sb = singles.tile([128, 1], bf16)
            nc.scalar.dma_start(out=wag_sb, in_=wag_in.ap())

            # features^T via xbar DMA-transpose (HWDGE, sync queue)
            fTk0 = singles.tile([128, N], bf16)
            fTk1 = singles.tile([128, N], bf16)
            fTr0 = singles.tile([128, ROWS], bf16)
            fTr1 = singles.tile([128, ROWS], bf16)
            nc.sync.dma_start_transpose(out=fTk0, in_=nfk_in.ap())
            nc.sync.dma_start_transpose(out=fTk1, in_=hidk_in.ap())
            for kh in range(KH):
                t = efp.tile([128, GPT, NS, KHW], bf16, tag="ef")
                nc.sync.dma_start(out=t, in_=T_in[:, kh, GPT:2 * GPT, :, :])
                ef_tiles[(1, kh)] = t
            nc.sync.dma_start_transpose(out=fTr0, in_=nfr_in.ap())
            nc.sync.dma_start_transpose(out=fTr1, in_=hidr_in.ap())

            # ---------------- phase 0: per-batch matmuls ----------------
            V = singles.tile([128, KC, O], bf16)
            for kc in range(KC):
                vps = psR.tile([128, O], fp32, tag="ret")
                ksl = slice(kc * 128, (kc + 1) * 128)
                nc.tensor.matmul(vps, fTk0[:, ksl], Wm_sb[:, 0, :],
                                 start=True, stop=False)
                nc.tensor.matmul(vps, fTk1[:, ksl], Wm_sb[:, 1, :],
                                 start=False, stop=False)
                nc.tensor.matmul(vps, ones_bf, bm_sb, start=False, stop=True)
                nc.vector.tensor_copy(V[:, kc, :], vps)

            att1_sb = singles.tile([128, RT], fp32)
            for rc in range(RT):
                aps = psR.tile([128, 1], fp32, tag="ret")
                rsl = slice(rc * 128, (rc + 1) * 128)
                nc.tensor.matmul(aps, fTr0[:, rsl], wa12_sb[:, 0:1],
                                 start=True, stop=False)
                nc.tensor.matmul(aps, fTr1[:, rsl], wa12_sb[:, 1:2],
                                 start=False, stop=True)
                nc.vector.tensor_copy(att1_sb[:, rc:rc + 1], aps)

            # att_g + sum(b_a*): sc = g@wag + bs -> bcast [128,1]
            gps = psR.tile([1, 1], fp32, tag="ret")
            nc.tensor.matmul(gps, g_sb, wag_sb, start=True, stop=True)
            sc1 = singles.tile([1, 1], fp32)
            nc.vector.tensor_copy(sc1, gps)
            nc.vector.tensor_scalar_add(sc1, sc1, bs_sb)
            sc1b = singles.tile([1, 1], bf16)
            nc.vector.tensor_copy(sc1b, sc1)
            scps = psR.tile([128, 1], fp32, tag="ret")
            nc.tensor.matmul(scps, ones_bf, sc1b, start=True, stop=True)
            sc128 = singles.tile([128, 1], fp32)
            nc.vector.tensor_copy(sc128, scps)
            att1g = singles.tile([128, RT], fp32)
            nc.vector.tensor_scalar_add(att1g, att1_sb, sc128)

            # att2 row over all keys: [1, N] bf16
            att2row = singles.tile([1, N], bf16)
            for khf in range(2):
                a2ps = psR.tile([1, 512], fp32, tag="ret")
                ksl = slice(khf * 512, (khf + 1) * 512)
                nc.tensor.matmul(a2ps, wa12_sb[:, 2:3], fTk0[:, ksl],
                                 start=True, stop=False)
                nc.tensor.matmul(a2ps, wa12_sb[:, 3:4], fTk1[:, ksl],
                                 start=False, stop=True)
                nc.vector.tensor_copy(att2row[:, ksl], a2ps)

            # ---------------- phase 1+2: staggered pipeline -------------
            sim_leaky = bool(os.environ.get("GAT_SIM_LEAKY"))
            ph2_state = {}

            def phase1(rt):
                rsl = slice(rt * 128, (rt + 1) * 128)
                nrt = rt + 1
                if nrt < RT and (nrt, 0) not in ef_tiles:
                    for kh in range(KH):
                        t = efp.tile([128, GPT, NS, KHW], bf16, tag="ef")
                        nc.sync.dma_start(
                            out=t,
                            in_=T_in[:, kh, nrt * GPT:(nrt + 1) * GPT, :, :])
                        ef_tiles[(nrt, kh)] = t
                if rt not in adj_tiles:
                    a = work.tile([128, N], bf16, tag="adj")
                    nc.scalar.dma_start(out=a, in_=adj_in[rsl, :])
                    adj_tiles[rt] = a
                adj_t = adj_tiles.pop(rt)

                lk = work.tile([128, N], bf16, tag="lk")
                for kh in range(KH):
                    ef_t = ef_tiles.pop((rt, kh))
                    hsl = slice(kh * KHW, (kh + 1) * KHW)
                    # PE: e-contraction + att2 assembly, PSUM fp32
                    lps = psL.tile([128, KHW], fp32, tag=f"l{kh}")
                    for gq in range(GPT):
                        for s in range(NS):
                            nc.tensor.matmul(
                                lps[32 * gq:32 * (gq + 1), :],
                                Wd_sb[:, s, :], ef_t[:, gq, s, :],
                                start=(s == 0), stop=False,
                                tile_position=(0, 32 * gq))
                    nc.tensor.matmul(lps, ones_bf, att2row[:, hsl],
                                     start=False, stop=True,
                                     skip_group_check=True)
                    # ScalarE: leaky_relu(x + att1g) PSUM -> SBUF bf16
                    if sim_leaky:
                        l2 = work.tile([128, KHW], fp32, tag=f"l2{kh}")
                        nc.vector.tensor_scalar_add(l2, lps,
                                                    att1g[:, rt:rt + 1])
                        nc.vector.scalar_tensor_tensor(
                            out=lk[:, hsl], in0=l2, scalar=0.01, in1=l2,
                            op0=ALU.mult, op1=ALU.max)
                    else:
                        nc.scalar.activation(lk[:, hsl], lps, AF.Lrelu,
                                             bias=att1g[:, rt:rt + 1],
                                             alpha=0.01)
                ex = work.tile([128, N], bf16, tag="ex")
                nc.scalar.activation(ex, lk, AF.Exp)
                # DVE: masked coefs + fused row-sum
                coefs = work.tile([128, N], bf16, tag="coefs")
                s = small.tile([128, 1], fp32, tag="s")
                nc.vector.scalar_tensor_tensor(
                    out=coefs, in0=ex, scalar=1.0, in1=adj_t,
                    op0=ALU.mult, op1=ALU.mult, accum_out=s)
                ph2_state[rt] = (coefs, s)

            def phase2(rt):
                rsl = slice(rt * 128, (rt + 1) * 128)
                coefs, s = ph2_state.pop(rt)
                if stage == 1:
                    o1 = work.tile([128, O], fp32, tag="outsb")
                    nc.vector.tensor_copy(o1, coefs[:, :O])
                    nc.sync.dma_start(out=out_t[rsl, :], in_=o1)
                    return
                ret_ps = psR.tile([128, O], fp32, tag="ret")
                for kc in range(KC):
                    tp = psT.tile([128, 128], bf16, tag="tp1")
                    nc.tensor.transpose(tp, coefs[:, kc * 128:(kc + 1) * 128],
                                        ident_sb)
                    ctT = small.tile([128, 128], bf16, tag="ctT")
                    nc.scalar.copy(out=ctT, in_=tp)
                    nc.tensor.matmul(ret_ps, ctT, V[:, kc, :],
                                     start=(kc == 0), stop=(kc == KC - 1))
                sk_ps = psT.tile([128, O], fp32, tag="skp")
                nc.tensor.matmul(sk_ps, fTr0[:, rsl], Wsk_sb[:, 0, :],
                                 start=True, stop=False)
                nc.tensor.matmul(sk_ps, fTr1[:, rsl], Wsk_sb[:, 1, :],
                                 start=False, stop=False)
                nc.tensor.matmul(sk_ps, ones_bf, bsk_sb,
                                 start=False, stop=True)
                r = small.tile([128, 1], fp32, tag="r")
                nc.vector.reciprocal(r, s)
                rets = small.tile([128, O], fp32, tag="rets")
                nc.scalar.mul(rets, ret_ps, r)
                sks = small.tile([128, O], fp32, tag="sks")
                nc.scalar.copy(out=sks, in_=sk_ps)
                out_sb = work.tile([128, O], fp32, tag="outsb")
                nc.vector.tensor_add(out_sb, rets, sks)
                nc.sync.dma_start(out=out_t[rsl, :], in_=out_sb)

            if stage >= 1:
                for rt in range(RT):
                    if rt >= 1:
                        phase2(rt - 1)
                    phase1(rt)
                phase2(RT - 1)
            else:
                for rt in range(RT):
                    o0 = work.tile([128, O], fp32, tag="outsb")
                    nc.vector.tensor_copy(o0, V[:, rt, :])
                    nc.sync.dma_start(out=out_t[rt * 128:(rt + 1) * 128, :],
                                      in_=o0)

    nc.compile()
    return nc


def _get_nc():
    if "nc" not in _cache:
        _cache["nc"] = _build(stage=int(os.environ.get("GAT_STAGE", "4")))
    return _cache["nc"]


def _in_maps(hidden, n_features, e_features, g_features, adj,
             W_m, b_m, W_skip, b_skip, w_a1, b_a1, w_a2, b_a2,
             w_ae, b_ae, w_ag, b_ag):
    import ml_dtypes
    bf16 = ml_dtypes.bfloat16
    f32 = np.float32
    asb = lambda x: np.ascontiguousarray(np.asarray(x).astype(bf16))
    bsum = (np.float32(np.asarray(b_a1).reshape(())) +
            np.float32(np.asarray(b_a2).reshape(())) +
            np.float32(np.asarray(b_ae).reshape(())) +
            np.float32(np.asarray(b_ag).reshape(())))
    wae = np.asarray(w_ae, f32).reshape(E)
    # Wd[p=(rm*4+es), s, rm'] = (rm==rm') * w_ae[4s+es]
    Wd = np.zeros((128, NS, 32), f32)
    for rm in range(32):
        for es in range(4):
            for s in range(NS):
                Wd[rm * 4 + es, s, rm] = wae[4 * s + es]
    wa12 = np.stack([np.asarray(w_a1, f32).reshape(2, 128)[0],
                     np.asarray(w_a1, f32).reshape(2, 128)[1],
                     np.asarray(w_a2, f32).reshape(2, 128)[0],
                     np.asarray(w_a2, f32).reshape(2, 128)[1]], axis=1)
    shared = {
        "Wm": asb(W_m), "bm": asb(b_m).reshape(1, O),
        "Wsk": asb(W_skip), "bsk": asb(b_skip).reshape(1, O),
        "wa12": asb(wa12), "Wd": asb(Wd), "wag": asb(w_ag),
        "bs": np.array([[bsum]], dtype=f32),
        "ident": np.eye(128, dtype=f32).astype(bf16),
    }
    maps = []
    adjm_all = {}
    for c in range(NCORES):
        b, h = c // 2, c % 2
        rows = slice(h * ROWS, (h + 1) * ROWS)
        if b not in adjm_all:
            adjm_all[b] = np.asarray(adj[b], f32).astype(bf16)
        m = dict(shared)
        # T[p=(rm*4+es), kh, G, s, k'] = ef[32G+rm, kh*512+k', 4s+es]
        efc = asb(e_features[b, rows])                    # [512,1024,16]
        T = efc.reshape(16, 32, KH, KHW, NS, 4)           # [G,rm,kh,k',s,es]
        T = T.transpose(1, 5, 2, 0, 4, 3)                 # [rm,es,kh,G,s,k']
        m["T"] = np.ascontiguousarray(T.reshape(128, KH, 16, NS, KHW))
        m["adj"] = np.ascontiguousarray(adjm_all[b][rows])
        m["nfk"] = asb(n_features[b])
        m["hidk"] = asb(hidden[b])
        m["nfr"] = asb(n_features[b][rows])
        m["hidr"] = asb(hidden[b][rows])
        m["g"] = asb(g_features[b]).reshape(G, 1)
        maps.append(m)
    return maps


def kernel(hidden, n_features, e_features, g_features, adj,
           W_m, b_m, W_skip, b_skip, w_a1, b_a1, w_a2, b_a2,
           w_ae, b_ae, w_ag, b_ag):
    from concourse import bass_utils
    nc = _get_nc()
    maps = _in_maps(hidden, n_features, e_features, g_features, adj,
                    W_m, b_m, W_skip, b_skip, w_a1, b_a1, w_a2, b_a2,
                    w_ae, b_ae, w_ag, b_ag)
    res = bass_utils.run_bass_kernel_spmd(nc, maps, core_ids=list(range(NCORES)))
    out = np.empty((B, N, O), np.float32)
    for c in range(NCORES):
        b, h = c // 2, c % 2
        out[b, h * ROWS:(h + 1) * ROWS] = res.results[c]["out"]
    return out


# revision 11
# speedup vs baseline: 1.6701x; 1.0556x over previous
"""GAT message-passing kernel for Trainium2, 8 NeuronCores.  (v5)

Problem (hardcoded): B=4, N=1024, H=F=O=G=128, E=16.
  features = concat([n_features, hidden], -1)            [B,N,256]
  values   = features @ W_m + b_m                        [B,N,128]
  logits   = att1 + att2^T + (e_features@w_ae) + att_g   [B,N,N]
  coefs    = softmax(leaky_relu(logits) + (adj-1)*1e9)
  out      = coefs @ values + features @ W_skip + b_skip

Sharding: 8 cores = (batch b = core//2) x (row half = core%2).

v5 strategy: the e-feature contraction runs entirely on the TENSOR engine.
  Host permutes ef into T[p=(rm*4+es), kh, g, s, k'] (rm=row%32, g=row
  group of 32, e=4s+es) so that a col-tiled matmul with a constant
  block-diagonal stationary Wd_s[128,32] (w_ae folded in) computes
  att_e for rows 32g..32g+31 directly at their natural partitions:
    lps[32g+rm', k] += sum_{rm,es} Wd_s[(rm,es),rm'] * T[(rm,es),g,s,k]
  PSUM accumulates att_e + adj mask + att2^T in fp32; att1+att_g ride the
  ScalarE leaky-relu bias port.  DVE does (almost) nothing per tile;
  GpSimd unused (its SBUF port contends with DVE).  DMA: bf16 host-cast,
  2 MiB k-half transfers on the sync HWDGE queue; small loads on scalar;
  features^T via xbar dma transpose.
"""

import os
import numpy as np

B, N, H, F, E, G, O = 4, 1024, 128, 128, 16, 128, 128
DIN = F + H
NCORES = 8
ROWS = N // 2          # query rows per core
RT = ROWS // 128       # row tiles per core
KC = N // 128          # key chunks
KH = 2                 # k halves
KHW = N // KH          # 512
GPT = 4                # 32-row groups per row tile
NS = 4                 # e-channel subgroups (4 channels each)

_cache = {}


def _build(stage=4):
    from contextlib import ExitStack
    import concourse.bacc as bacc
    import concourse.tile as tile
    import concourse.mybir as mybir
    import concourse.bass as bass

    fp32 = mybir.dt.float32
    bf16 = mybir.dt.bfloat16
    ALU = mybir.AluOpType
    AF = mybir.ActivationFunctionType

    nc = bacc.Bacc("TRN2", target_bir_lowering=False, debug=False,
                   num_devices=NCORES)

    # T layout: [p=(rm*4+es), kh, G, s, k'] ; G = global 32-row group
    T_in = nc.dram_tensor("T", [128, KH, 16, NS, KHW], bf16,
                          kind="ExternalInput")
    Wd_in = nc.dram_tensor("Wd", [128, NS, 32], bf16, kind="ExternalInput")
    adj_in = nc.dram_tensor("adj", [ROWS, N], bf16, kind="ExternalInput")
    nfk_in = nc.dram_tensor("nfk", [N, F], bf16, kind="ExternalInput")
    hidk_in = nc.dram_tensor("hidk", [N, H], bf16, kind="ExternalInput")
    nfr_in = nc.dram_tensor("nfr", [ROWS, F], bf16, kind="ExternalInput")
    hidr_in = nc.dram_tensor("hidr", [ROWS, H], bf16, kind="ExternalInput")
    g_in = nc.dram_tensor("g", [G, 1], bf16, kind="ExternalInput")
    Wm_in = nc.dram_tensor("Wm", [DIN, O], bf16, kind="ExternalInput")
    bm_in = nc.dram_tensor("bm", [1, O], bf16, kind="ExternalInput")
    Wsk_in = nc.dram_tensor("Wsk", [DIN, O], bf16, kind="ExternalInput")
    bsk_in = nc.dram_tensor("bsk", [1, O], bf16, kind="ExternalInput")
    wa12_in = nc.dram_tensor("wa12", [128, 4], bf16, kind="ExternalInput")
    wag_in = nc.dram_tensor("wag", [G, 1], bf16, kind="ExternalInput")
    bs_in = nc.dram_tensor("bs", [1, 1], fp32, kind="ExternalInput")
    ident_in = nc.dram_tensor("ident", [128, 128], bf16, kind="ExternalInput")
    out_t = nc.dram_tensor("out", [ROWS, O], fp32, kind="ExternalOutput")

    with tile.TileContext(nc) as tc:
        with ExitStack() as ctx:
            singles = ctx.enter_context(tc.tile_pool(name="singles", bufs=1))
            efp = ctx.enter_context(tc.tile_pool(name="efp", bufs=6))
            work = ctx.enter_context(tc.tile_pool(name="work", bufs=2))
            small = ctx.enter_context(tc.tile_pool(name="small", bufs=2))
            psL = ctx.enter_context(tc.tile_pool(name="psL", bufs=1, space="PSUM"))
            psT = ctx.enter_context(tc.tile_pool(name="psT", bufs=2, space="PSUM"))
            psR = ctx.enter_context(tc.tile_pool(name="psR", bufs=2, space="PSUM"))

            # ---- prime the ef stream (split first transfer for fast start)
            ef_tiles = {}
            t00 = efp.tile([128, GPT, NS, KHW], bf16, tag="ef")
            nc.sync.dma_start(out=t00[:, 0:2, :, :], in_=T_in[:, 0, 0:2, :, :])
            nc.sync.dma_start(out=t00[:, 2:4, :, :], in_=T_in[:, 0, 2:4, :, :])
            ef_tiles[(0, 0)] = t00
            t01 = efp.tile([128, GPT, NS, KHW], bf16, tag="ef")
            nc.sync.dma_start(out=t01, in_=T_in[:, 1, 0:GPT, :, :])
            ef_tiles[(0, 1)] = t01

            # ---- constants (scalar HWDGE queue) ------------------------
            Wd_sb = singles.tile([128, NS, 32], bf16)
            nc.scalar.dma_start(out=Wd_sb, in_=Wd_in.ap())
            adj_tiles = {}
            adj0 = work.tile([128, N], bf16, tag="adj")
            nc.scalar.dma_start(out=adj0, in_=adj_in[0:128, :])
            adj_tiles[0] = adj0
            ident_sb = singles.tile([128, 128], bf16)
            nc.scalar.dma_start(out=ident_sb, in_=ident_in.ap())
            ones_bf = singles.tile([1, 128], bf16)
            nc.vector.memset(ones_bf, 1.0)

            Wm_sb = singles.tile([128, 2, O], bf16)
            Wsk_sb = singles.tile([128, 2, O], bf16)
            for c in range(2):
                nc.scalar.dma_start(out=Wm_sb[:, c, :],
                                    in_=Wm_in[c * 128:(c + 1) * 128, :])
                nc.scalar.dma_start(out=Wsk_sb[:, c, :],
                                    in_=Wsk_in[c * 128:(c + 1) * 128, :])
            wa12_sb = singles.tile([128, 4], bf16)
            nc.scalar.dma_start(out=wa12_sb, in_=wa12_in.ap())
            bm_sb = singles.tile([1, O], bf16)
            nc.scalar.dma_start(out=bm_sb, in_=bm_in.ap())
            bsk_sb = singles.tile([1, O], bf16)
            nc.scalar.dma_start(out=bsk_sb, in_=bsk_in.ap())
            bs_sb = singles.tile([1, 1], fp32)
            nc.scalar.dma_start(out=bs_sb, in_=bs_in.ap())
            g_sb = singles.tile([128, 1], bf16)
            nc.scalar.dma_start(out=g_sb, in_=g_in.ap())
            wag_sb = singles.tile([128, 1], bf16)
            nc.scalar.dma_start(out=wag_sb, in_=wag_in.ap())

            # PE warmup spin: sustained activity flips HAM to 8/8 early
            wups = psL.tile([128, 32], fp32, tag="l0")
            for i in range(50):
                nc.tensor.matmul(wups[0:32, :], Wd_sb[:, 0, :],
                                 Wd_sb[:, 1, :], start=True, stop=True,
                                 tile_position=(0, 0),
                                 skip_group_check=True)

            # features^T via xbar DMA-transpose (HWDGE, sync queue)
            fTk0 = singles.tile([128, N], bf16)
            fTk1 = singles.tile([128, N], bf16)
            fTr0 = singles.tile([128, ROWS], bf16)
            fTr1 = singles.tile([128, ROWS], bf16)
            nc.sync.dma_start_transpose(out=fTk0, in_=nfk_in.ap())
            nc.sync.dma_start_transpose(out=fTk1, in_=hidk_in.ap())
            for kh in range(KH):
                t = efp.tile([128, GPT, NS, KHW], bf16, tag="ef")
                nc.sync.dma_start(out=t, in_=T_in[:, kh, GPT:2 * GPT, :, :])
                ef_tiles[(1, kh)] = t
            nc.sync.dma_start_transpose(out=fTr0, in_=nfr_in.ap())
            nc.sync.dma_start_transpose(out=fTr1, in_=hidr_in.ap())

            # ---------------- phase 0: per-batch matmuls ----------------
            V = singles.tile([128, KC, O], bf16)
            for kc in range(KC):
                vps = psR.tile([128, O], fp32, tag="ret")
                ksl = slice(kc * 128, (kc + 1) * 128)
                nc.tensor.matmul(vps, fTk0[:, ksl], Wm_sb[:, 0, :],
                                 start=True, stop=False)
                nc.tensor.matmul(vps, fTk1[:, ksl], Wm_sb[:, 1, :],
                                 start=False, stop=False)
                nc.tensor.matmul(vps, ones_bf, bm_sb, start=False, stop=True)
                nc.vector.tensor_copy(V[:, kc, :], vps)

            att1_sb = singles.tile([128, RT], fp32)
            for rc in range(RT):
                aps = psR.tile([128, 1], fp32, tag="ret")
                rsl = slice(rc * 128, (rc + 1) * 128)
                nc.tensor.matmul(aps, fTr0[:, rsl], wa12_sb[:, 0:1],
                                 start=True, stop=False)
                nc.tensor.matmul(aps, fTr1[:, rsl], wa12_sb[:, 1:2],
                                 start=False, stop=True)
                nc.vector.tensor_copy(att1_sb[:, rc:rc + 1], aps)

            # att_g + sum(b_a*): sc = g@wag + bs -> bcast [128,1]
            gps = psR.tile([1, 1], fp32, tag="ret")
            nc.tensor.matmul(gps, g_sb, wag_sb, start=True, stop=True)
            sc1 = singles.tile([1, 1], fp32)
            nc.vector.tensor_copy(sc1, gps)
            nc.vector.tensor_scalar_add(sc1, sc1, bs_sb)
            sc1b = singles.tile([1, 1], bf16)
            nc.vector.tensor_copy(sc1b, sc1)
            scps = psR.tile([128, 1], fp32, tag="ret")
            nc.tensor.matmul(scps, ones_bf, sc1b, start=True, stop=True)
            sc128 = singles.tile([128, 1], fp32)
            nc.vector.tensor_copy(sc128, scps)
            att1g = singles.tile([128, RT], fp32)
            nc.vector.tensor_scalar_add(att1g, att1_sb, sc128)

            # att2 row over all keys: [1, N] bf16
            att2row = singles.tile([1, N], bf16)
            for khf in range(2):
                a2ps = psR.tile([1, 512], fp32, tag="ret")
                ksl = slice(khf * 512, (khf + 1) * 512)
                nc.tensor.matmul(a2ps, wa12_sb[:, 2:3], fTk0[:, ksl],
                                 start=True, stop=False)
                nc.tensor.matmul(a2ps, wa12_sb[:, 3:4], fTk1[:, ksl],
                                 start=False, stop=True)
                nc.vector.tensor_copy(att2row[:, ksl], a2ps)

            # ---------------- phase 1+2: staggered pipeline -------------
            sim_leaky = bool(os.environ.get("GAT_SIM_LEAKY"))
            ph2_state = {}

            def phase1(rt):
                rsl = slice(rt * 128, (rt + 1) * 128)
                nrt = rt + 1
                if nrt < RT and (nrt, 0) not in ef_tiles:
                    for kh in range(KH):
                        t = efp.tile([128, GPT, NS, KHW], bf16, tag="ef")
                        nc.sync.dma_start(
                            out=t,
                            in_=T_in[:, kh, nrt * GPT:(nrt + 1) * GPT, :, :])
                        ef_tiles[(nrt, kh)] = t
                if rt not in adj_tiles:
                    a = work.tile([128, N], bf16, tag="adj")
                    nc.scalar.dma_start(out=a, in_=adj_in[rsl, :])
                    adj_tiles[rt] = a
                adj_t = adj_tiles.pop(rt)

                lk = work.tile([128, N], bf16, tag="lk")
                for kh in range(KH):
                    ef_t = ef_tiles.pop((rt, kh))
                    hsl = slice(kh * KHW, (kh + 1) * KHW)
                    # PE: e-contraction + att2 assembly, PSUM fp32
                    lps = psL.tile([128, KHW], fp32, tag=f"l{kh}")
                    for gq in range(GPT):
                        for s in range(NS):
                            nc.tensor.matmul(
                                lps[32 * gq:32 * (gq + 1), :],
                                Wd_sb[:, s, :], ef_t[:, gq, s, :],
                                start=(s == 0), stop=False,
                                tile_position=(0, 32 * gq))
                    nc.tensor.matmul(lps, ones_bf, att2row[:, hsl],
                                     start=False, stop=True,
                                     skip_group_check=True)
                    # ScalarE: leaky_relu(x + att1g) PSUM -> SBUF bf16
                    if sim_leaky:
                        l2 = work.tile([128, KHW], fp32, tag=f"l2{kh}")
                        nc.vector.tensor_scalar_add(l2, lps,
                                                    att1g[:, rt:rt + 1])
                        nc.vector.scalar_tensor_tensor(
                            out=lk[:, hsl], in0=l2, scalar=0.01, in1=l2,
                            op0=ALU.mult, op1=ALU.max)
                    else:
                        nc.scalar.activation(lk[:, hsl], lps, AF.Lrelu,
                                             bias=att1g[:, rt:rt + 1],
                                             alpha=0.01)
                ex = work.tile([128, N], bf16, tag="ex")
                nc.scalar.activation(ex, lk, AF.Exp)
                # DVE: masked coefs + fused row-sum
                coefs = work.tile([128, N], bf16, tag="coefs")
                s = small.tile([128, 1], fp32, tag="s")
                nc.vector.scalar_tensor_tensor(
                    out=coefs, in0=ex, scalar=1.0, in1=adj_t,
                    op0=ALU.mult, op1=ALU.mult, accum_out=s)
                ph2_state[rt] = (coefs, s)

            def phase2(rt):
                rsl = slice(rt * 128, (rt + 1) * 128)
                coefs, s = ph2_state.pop(rt)
                if stage == 1:
                    o1 = work.tile([128, O], fp32, tag="outsb")
                    nc.vector.tensor_copy(o1, coefs[:, :O])
                    nc.sync.dma_start(out=out_t[rsl, :], in_=o1)
                    return
                ret_ps = psR.tile([128, O], fp32, tag="ret")
                for kc in range(KC):
                    tp = psT.tile([128, 128], bf16, tag="tp1")
                    nc.tensor.transpose(tp, coefs[:, kc * 128:(kc + 1) * 128],
                                        ident_sb)
                    ctT = small.tile([128, 128], bf16, tag="ctT")
                    nc.scalar.copy(out=ctT, in_=tp)
                    nc.tensor.matmul(ret_ps, ctT, V[:, kc, :],
                                     start=(kc == 0), stop=(kc == KC - 1))
                sk_ps = psT.tile([128, O], fp32, tag="skp")
                nc.tensor.matmul(sk_ps, fTr0[:, rsl], Wsk_sb[:, 0, :],
                                 start=True, stop=False)
                nc.tensor.matmul(sk_ps, fTr1[:, rsl], Wsk_sb[:, 1, :],
                                 start=False, stop=False)
                nc.tensor.matmul(sk_ps, ones_bf, bsk_sb,
                                 start=False, stop=True)
                r = small.tile([128, 1], fp32, tag="r")
                nc.vector.reciprocal(r, s)
                rets = small.tile([128, O], fp32, tag="rets")
                nc.scalar.mul(rets, ret_ps, r)
                sks = small.tile([128, O], fp32, tag="sks")
                nc.scalar.copy(out=sks, in_=sk_ps)
                out_sb = work.tile([128, O], fp32, tag="outsb")
                nc.vector.tensor_add(out_sb, rets, sks)
                nc.sync.dma_start(out=out_t[rsl, :], in_=out_sb)

            if stage >= 1:
                for rt in range(RT):
                    phase1(rt)
                    if rt >= 1:
                        phase2(rt - 1)
                phase2(RT - 1)
            else:
                for rt in range(RT):
                    o0 = work.tile([128, O], fp32, tag="outsb")
                    nc.vector.tensor_copy(o0, V[:, rt, :])
                    nc.sync.dma_start(out=out_t[rt * 128:(rt + 1) * 128, :],
                                      in_=o0)

    nc.compile()
    return nc


def _get_nc():
    if "nc" not in _cache:
        _cache["nc"] = _build(stage=int(os.environ.get("GAT_STAGE", "4")))
    return _cache["nc"]


def _in_maps(hidden, n_features, e_features, g_features, adj,
             W_m, b_m, W_skip, b_skip, w_a1, b_a1, w_a2, b_a2,
             w_ae, b_ae, w_ag, b_ag):
    import ml_dtypes
    bf16 = ml_dtypes.bfloat16
    f32 = np.float32
    asb = lambda x: np.ascontiguousarray(np.asarray(x).astype(bf16))
    bsum = (np.float32(np.asarray(b_a1).reshape(())) +
            np.float32(np.asarray(b_a2).reshape(())) +
            np.float32(np.asarray(b_ae).reshape(())) +
            np.float32(np.asarray(b_ag).reshape(())))
    wae = np.asarray(w_ae, f32).reshape(E)
    # Wd[p=(rm*4+es), s, rm'] = (rm==rm') * w_ae[4s+es]
    Wd = np.zeros((128, NS, 32), f32)
    for rm in range(32):
        for es in range(4):
            for s in range(NS):
                Wd[rm * 4 + es, s, rm] = wae[4 * s + es]
    wa12 = np.stack([np.asarray(w_a1, f32).reshape(2, 128)[0],
                     np.asarray(w_a1, f32).reshape(2, 128)[1],
                     np.asarray(w_a2, f32).reshape(2, 128)[0],
                     np.asarray(w_a2, f32).reshape(2, 128)[1]], axis=1)
    shared = {
        "Wm": asb(W_m), "bm": asb(b_m).reshape(1, O),
        "Wsk": asb(W_skip), "bsk": asb(b_skip).reshape(1, O),
        "wa12": asb(wa12), "Wd": asb(Wd), "wag": asb(w_ag),
        "bs": np.array([[bsum]], dtype=f32),
        "ident": np.eye(128, dtype=f32).astype(bf16),
    }
    maps = []
    adjm_all = {}
    for c in range(NCORES):
        b, h = c // 2, c % 2
        rows = slice(h * ROWS, (h + 1) * ROWS)
        if b not in adjm_all:
            adjm_all[b] = np.asarray(adj[b], f32).astype(bf16)
        m = dict(shared)
        # T[p=(rm*4+es), kh, G, s, k'] = ef[32G+rm, kh*512+k', 4s+es]
        efc = asb(e_features[b, rows])                    # [512,1024,16]
        T = efc.reshape(16, 32, KH, KHW, NS, 4)           # [G,rm,kh,k',s,es]
        T = T.transpose(1, 5, 2, 0, 4, 3)                 # [rm,es,kh,G,s,k']
        m["T"] = np.ascontiguousarray(T.reshape(128, KH, 16, NS, KHW))
        m["adj"] = np.ascontiguousarray(adjm_all[b][rows])
        m["nfk"] = asb(n_features[b])
        m["hidk"] = asb(hidden[b])
        m["nfr"] = asb(n_features[b][rows])
        m["hidr"] = asb(hidden[b][rows])
        m["g"] = asb(g_features[b]).reshape(G, 1)
        maps.append(m)
    return maps


def kernel(hidden, n_features, e_features, g_features, adj,
           W_m, b_m, W_skip, b_skip, w_a1, b_a1, w_a2, b_a2,
           w_ae, b_ae, w_ag, b_ag):
    from concourse import bass_utils
    nc = _get_nc()
    maps = _in_maps(hidden, n_features, e_features, g_features, adj,
                    W_m, b_m, W_skip, b_skip, w_a1, b_a1, w_a2, b_a2,
                    w_ae, b_ae, w_ag, b_ag)
    res = bass_utils.run_bass_kernel_spmd(nc, maps, core_ids=list(range(NCORES)))
    out = np.empty((B, N, O), np.float32)
    for c in range(NCORES):
        b, h = c // 2, c % 2
        out[b, h * ROWS:(h + 1) * ROWS] = res.results[c]["out"]
    return out
